# revision 13
# baseline (speedup 1.0000x reference)
"""Trainium2 Bass kernel for nn_AttentionBlock (B=2, N=2048, dim=1024, 16 heads x 64).

Sharding: 8 cores = 2 batches x 4 head-groups (4 heads per core, tensor-parallel
over heads for qkv/attention; the to_out projection is computed as per-core,
per-i-tile partial sums gathered and added on host).

Per-core device program (SPMD, identical shapes on every core):
  inputs (bf16, pre-transposed on host):
    xT [1024, 2048], wqT/wkT/wvT [1024, 256], woT [256, 1024]
  outputs (f32): y0, y1 [2048, 1024] — partial projections for i-tile 0
    (heads 0,1) and i-tile 1 (heads 2,3); host adds them.

Structure: per head-pair (= i-tile) and 512-wide q-window, a 16-step loop over
k-tiles computes S^T for both heads concurrently (row-groups 0-63 / 64-127 of
the PE array, one [128, 2, 512] PSUM tile), one exp ACTIVATE (FD=1024, fused
1/8 scale, PSUM->SBUF bf16), then P^T @ [V|1] accumulates O^T plus softmax row
sums in PSUM. Normalize+project for each step is emitted one step later so its
reciprocal DMA chain never stalls the PE queue. Matmuls are bf16 with fp32
accumulation; softmax skips max-subtraction (logits ~N(0,1), exp safe in fp32).
"""

import ml_dtypes
import numpy as np

import concourse.bass as bass
import concourse.mybir as mybir
import concourse.tile as tile
from concourse.bass_utils import run_bass_kernel_spmd

B = 2
N = 2048
D = 1024
H = 16
DH = 64
HPC = 4  # heads per core
NCORES = 8
HB = HPC * DH  # 256: head-block width per core
NKT = N // 128  # 16 k-tiles
NW = 4  # 512-wide q-windows

f32 = mybir.dt.float32
f32r = mybir.dt.float32r
bf16 = mybir.dt.bfloat16
EXP = mybir.ActivationFunctionType.Exp

_WAIT_CAP = 1


def _split_excess_waits(nc):
    """The walrus build in this container rejects instructions carrying more
    than a couple of sync-wait commands ("Too many sync wait commands" in
    CoreV3GenImpl setupSyncWait). Tile's semaphore assignment freely attaches
    several waits to one instruction. Hoist the excess onto dedicated
    single-wait NOPs inserted just before the instruction on the same engine
    (program order on that engine preserves the wait-before-execute
    semantics)."""
    f = nc.m.functions[0]
    for blk in f.blocks:
        out = []
        changed = False
        for inst in blk.instructions:
            si = inst.sync_info
            waits = list(si.on_wait) if si is not None and si.on_wait else []
            if len(waits) > _WAIT_CAP:
                changed = True
                for j, w in enumerate(waits[: -_WAIT_CAP]):
                    nop = mybir.InstNoOp(
                        name=f"{inst.name}-ws{j}",
                        engine=inst.engine,
                        sync_info=mybir.SyncInfo(on_wait=[w], on_update=[]),
                        bass_nofuse=True,
                    )
                    nc.register_instruction(nop)
                    out.append(nop)
                si.on_wait = waits[-_WAIT_CAP:]
            out.append(inst)
        if changed:
            blk.instructions = out


def _r(ap):
    return ap.bitcast(f32r)


def _build_nc():
    nc = bass.Bass()
    xT_d = nc.dram_tensor("xT", [D, N], bf16, kind="ExternalInput")
    wqT_d = nc.dram_tensor("wqT", [D, HB], bf16, kind="ExternalInput")
    wkT_d = nc.dram_tensor("wkT", [D, HB], bf16, kind="ExternalInput")
    wvT_d = nc.dram_tensor("wvT", [D, HB], bf16, kind="ExternalInput")
    woT_d = nc.dram_tensor("woT", [HB, D], bf16, kind="ExternalInput")
    y_ds = [
        nc.dram_tensor(f"y{it}", [N, D], f32, kind="ExternalOutput") for it in range(2)
    ]

    with tile.TileContext(nc) as tc:
        with (
            tc.tile_pool(name="main", bufs=1) as main,
            tc.tile_pool(name="ptp", bufs=3) as ptp,
            tc.tile_pool(name="ysp", bufs=3) as ysp,
            tc.tile_pool(name="spp", bufs=2) as spp,
            tc.tile_pool(name="drm", bufs=2, space="DRAM") as drm,
            tc.tile_pool(name="aux", bufs=2, space="PSUM") as aux,
            tc.tile_pool(name="stp", bufs=2, space="PSUM") as stp,
            tc.tile_pool(name="otp", bufs=1, space="PSUM") as otp,
        ):
            # persistent tensors
            qT = main.tile([128, 2, N], bf16)  # row d = it*128+p
            kT = main.tile([128, 2, N], bf16)
            vaug = main.tile([128, NKT, HPC, DH + 1], bf16)  # [k%128, k//128, h, d|1]
            ocat = main.tile([128, 2, N], bf16)  # row i = it*128+p
            sel = main.tile([2, 128], f32)
            wo = main.tile([128, 2, D], bf16)
            xt = main.tile([128, 8, N], bf16)
            wq = main.tile([128, 8, HB], bf16)
            wk = main.tile([128, 8, HB], bf16)
            wv = main.tile([128, 8, HB], bf16)

            sel_np = np.zeros((2, 128), dtype=np.float32)
            sel_np[0, 0:64] = 1.0
            sel_np[1, 64:128] = 1.0
            sel_d = nc.inline_tensor(sel_np, name="sel_const")
            nc.sync.dma_start(_r(sel[:]), _r(sel_d[:]))
            ones_t = main.tile([128, 1], bf16)
            nc.vector.memset(ones_t[:], 1.0)
            nc.vector.tensor_copy(
                vaug[:, :, :, DH : DH + 1],
                ones_t[:, :, None, None].to_broadcast([128, NKT, HPC, 1]),
            )
            for eo in range(8):
                sl = slice(eo * 128, (eo + 1) * 128)
                nc.sync.dma_start(wk[:, eo], wkT_d[sl])
                nc.gpsimd.dma_start(wq[:, eo], wqT_d[sl])
            for eo in range(8):
                sl = slice(eo * 128, (eo + 1) * 128)
                nc.sync.dma_start(xt[:, eo], xT_d[sl])
                nc.gpsimd.dma_start(wv[:, eo], wvT_d[sl])
            for it in range(2):
                nc.gpsimd.dma_start(wo[:, it], woT_d[it * 128 : (it + 1) * 128])

            # ---- projection-group emitters (each: 8 accumulating matmuls) ----
            def emit_qk_group(dst, w, it, q4):
                ps = aux.tile([128, 512], f32, tag="aux")
                for eo in range(8):
                    nc.tensor.matmul(
                        ps,
                        lhsT=w[:, eo, it * 128 : (it + 1) * 128],
                        rhs=xt[:, eo, q4 * 512 : (q4 + 1) * 512],
                        start=(eo == 0),
                        stop=(eo == 7),
                    )
                nc.vector.tensor_copy(dst[:, it, q4 * 512 : (q4 + 1) * 512], ps)

            def emit_v_group(nt):
                ps_full = aux.tile([128, 512], f32, tag="aux", name=f"vps{nt}")
                ps = ps_full[:, 0:HB]
                for eo in range(8):
                    nc.tensor.matmul(
                        ps,
                        lhsT=xt[:, eo, nt * 128 : (nt + 1) * 128],
                        rhs=wv[:, eo, :],
                        start=(eo == 0),
                        stop=(eo == 7),
                    )
                nc.vector.tensor_copy(
                    vaug[:, nt, :, 0:DH], ps.rearrange("p (h d) -> p h d", h=HPC)
                )

            # upfront groups: what the first attention step (pair 0, window 0)
            # consumes early: kT it0 (read across all k-tiles) and qT it0 w0.
            emit_qk_group(kT, wk, 0, 0)
            emit_qk_group(qT, wq, 0, 0)
            for q4 in range(1, 4):
                emit_qk_group(kT, wk, 0, q4)

            # remaining projection groups, drip-fed into attention steps.
            # IMPORTANT: Tile tracks dependencies in emission order, so every
            # group must be EMITTED strictly before the step that consumes it
            # (step s consumes qT[it=s//4] window q4=s%4; kT[it=1] is consumed
            # from step 4 on). Step 0 also interleaves the 16 v-groups just
            # ahead of their PV consumers.
            drip = {
                0: [lambda: emit_qk_group(qT, wq, 0, 1)],
                1: [lambda: emit_qk_group(qT, wq, 0, 2)]
                + [lambda q4=q4: emit_qk_group(kT, wk, 1, q4) for q4 in range(0, 3)],
                2: [
                    lambda: emit_qk_group(qT, wq, 0, 3),
                    lambda: emit_qk_group(kT, wk, 1, 3),
                    lambda: emit_qk_group(qT, wq, 1, 0),
                ],
                3: [lambda: emit_qk_group(qT, wq, 1, 1)],
                4: [lambda: emit_qk_group(qT, wq, 1, 2)],
                5: [lambda: emit_qk_group(qT, wq, 1, 3)],
            }

            # ---- attention steps ----
            steps = [(it, w) for it in range(2) for w in range(NW)]

            def att_step(step_idx, it, w, mid=None):
                q0 = w * 512
                h_lo, h_hi = 2 * it, 2 * it + 1
                ot_lo = otp.tile([128, 512], f32, tag="otlo")
                ot_hi = otp.tile([128, 512], f32, tag="othi")
                fillers = list(drip.get(step_idx, []))
                for kt in range(NKT):
                    if mid is not None and kt == 2:
                        mid()
                        mid = None
                    if step_idx == 0:
                        emit_v_group(kt)
                    elif fillers and kt % 4 == 1:
                        fillers.pop(0)()
                    st2 = stp.tile([128, 2, 512], f32, tag="st")
                    for s in range(2):
                        nc.tensor.matmul(
                            st2[:, s, :],
                            lhsT=kT[
                                s * 64 : s * 64 + 64, it, kt * 128 : (kt + 1) * 128
                            ],
                            rhs=qT[s * 64 : s * 64 + 64, it, q0 : q0 + 512],
                            start=True,
                            stop=True,
                        )
                    pt2 = ptp.tile([128, 2, 512], bf16, tag="pt")
                    nc.scalar.activation(
                        pt2.rearrange("p s q -> p (s q)"),
                        st2.rearrange("p s q -> p (s q)"),
                        EXP,
                        scale=0.125,
                    )
                    for s, ot in ((0, ot_lo), (1, ot_hi)):
                        nc.tensor.matmul(
                            ot[0:65, :],
                            lhsT=vaug[:, kt, 2 * it + s, :],
                            rhs=pt2[:, s, :],
                            start=(kt == 0),
                            stop=(kt == NKT - 1),
                        )
                if mid is not None:
                    mid()
                while fillers:
                    fillers.pop(0)()
                # evacuate: unnormalized O^T (cast to bf16) + row sums
                sst = spp.tile([1, 1024], f32, tag="sst")
                for s, ot in ((0, ot_lo), (1, ot_hi)):
                    nc.vector.tensor_copy(
                        ocat[s * 64 : s * 64 + 64, it, q0 : q0 + 512], ot[0:64, :]
                    )
                    nc.vector.tensor_copy(
                        sst[0:1, s * 512 : (s + 1) * 512], ot[64:65, :]
                    )
                return sst

            def np_step(it, w, sst):
                q0 = w * 512
                # reciprocal of the row sums, spread over 128 partitions via a
                # DRAM bounce (SBUF partition dim can't be reshaped in place)
                stmp = drm.tile([1, 1024], f32, tag="stmp")
                nc.sync.dma_start(stmp, sst)
                spk = spp.tile([128, 8], f32, tag="spk")
                nc.sync.dma_start(spk, stmp.rearrange("a (p j) -> (a p) j", p=128))
                rpk = spp.tile([128, 8], f32, tag="rpk")
                nc.vector.reciprocal(rpk, spk)
                rtmp = drm.tile([128, 8], f32, tag="rtmp")
                nc.sync.dma_start(rtmp, rpk)
                rq = spp.tile([2, 512], f32, tag="rq")
                nc.sync.dma_start(
                    _r(rq[:]),
                    _r(rtmp.rearrange("p j -> (p j)").rearrange("(s q) -> s q", s=2)),
                )
                # broadcast 1/sum across each head's 64 rows and normalize
                bc = aux.tile([128, 512], f32, tag="aux")
                nc.tensor.matmul(bc, lhsT=_r(sel), rhs=_r(rq), start=True, stop=True)
                bcb = ysp.tile([128, 512], bf16, tag="bcb")
                nc.vector.tensor_copy(bcb[:], bc)
                osl = ocat[:, it, q0 : q0 + 512]
                nc.vector.tensor_mul(osl, osl, bcb)
                # output projection for this (i-tile, window): partial sums
                for qt in range(4):
                    r0 = q0 + qt * 128
                    for oc in range(2):
                        yp = aux.tile([128, 512], f32, tag="aux")
                        nc.tensor.matmul(
                            yp,
                            lhsT=ocat[:, it, r0 : r0 + 128],
                            rhs=wo[:, it, oc * 512 : (oc + 1) * 512],
                            start=True,
                            stop=True,
                        )
                        ys = ysp.tile([128, 512], f32, tag="ys")
                        nc.vector.tensor_copy(ys, yp)
                        nc.sync.dma_start(
                            y_ds[it][r0 : r0 + 128, oc * 512 : (oc + 1) * 512], ys
                        )

            prev_np = None  # previous step's norm+project, emitted mid next step
            for idx, (it, w) in enumerate(steps):
                sst = att_step(idx, it, w, mid=prev_np)
                prev_np = (
                    lambda it=it, w=w, sst=sst: np_step(it, w, sst)
                )
            prev_np()  # the last step's norm+project is the kernel tail

    _split_excess_waits(nc)
    return nc


_CACHED_NC = None


def _get_nc():
    global _CACHED_NC
    if _CACHED_NC is None:
        _CACHED_NC = _build_nc()
    return _CACHED_NC


def _make_in_maps(x, w_qkv):
    b16 = ml_dtypes.bfloat16

    def c(a):
        return np.ascontiguousarray(a.astype(b16))

    in_maps = []
    xT = [c(x[b].T) for b in range(B)]
    for core in range(NCORES):
        b = core // (NCORES // B)
        hb = core % (NCORES // B)
        rows = slice(hb * HB, (hb + 1) * HB)
        wq = c(w_qkv[0 * D : 1 * D][rows].T)
        wk = c(w_qkv[1 * D : 2 * D][rows].T)
        wv = c(w_qkv[2 * D : 3 * D][rows].T)
        in_maps.append({"xT": xT[b], "wqT": wq, "wkT": wk, "wvT": wv})
    return in_maps


def kernel(x, w_qkv, w_out, b_out, _trace=False, _trace_kwargs=None):
    x = np.asarray(x, dtype=np.float32)
    w_qkv = np.asarray(w_qkv, dtype=np.float32)
    w_out = np.asarray(w_out, dtype=np.float32)
    b_out = np.asarray(b_out, dtype=np.float32)

    in_maps = _make_in_maps(x, w_qkv)
    for core in range(NCORES):
        hb = core % (NCORES // B)
        woT = np.ascontiguousarray(
            w_out[:, hb * HB : (hb + 1) * HB].T.astype(ml_dtypes.bfloat16)
        )
        in_maps[core]["woT"] = woT

    nc = _get_nc()
    kwargs = {}
    if _trace:
        kwargs["trace"] = True
        if _trace_kwargs:
            kwargs.update(_trace_kwargs)
    res = run_bass_kernel_spmd(nc, in_maps, core_ids=list(range(NCORES)), **kwargs)

    out = np.zeros((B, N, D), dtype=np.float32)
    for core in range(NCORES):
        b = core // (NCORES // B)
        out[b] += res.results[core]["y0"]
        out[b] += res.results[core]["y1"]
    out += b_out[None, None, :]
    kernel._last_result = res
    return out


# revision 14
# speedup vs baseline: 1.1109x; 1.1109x over previous
"""Trainium2 Bass kernel for nn_AttentionBlock (B=2, N=2048, dim=1024, 16 heads x 64).

Sharding: 8 cores = 2 batches x 4 head-groups (4 heads per core, tensor-parallel
over heads for qkv/attention; the to_out projection is computed as per-core,
per-i-tile partial sums gathered and added on host).

Per-core device program (SPMD, identical shapes on every core):
  inputs (bf16, pre-transposed on host):
    xT [1024, 2048], wqT/wkT/wvT [1024, 256], woT [256, 1024]
  outputs (f32): y0, y1 [2048, 1024] — partial projections for i-tile 0
    (heads 0,1) and i-tile 1 (heads 2,3); host adds them.

Structure: per head-pair (= i-tile) and 512-wide q-window, a 16-step loop over
k-tiles computes S^T for both heads concurrently (row-groups 0-63 / 64-127 of
the PE array, one [128, 2, 512] PSUM tile), one exp ACTIVATE (FD=1024, fused
1/8 scale, PSUM->SBUF bf16), then P^T @ [V|1] accumulates O^T plus softmax row
sums in PSUM. Normalize+project for each step is emitted one step later so its
reciprocal DMA chain never stalls the PE queue. Matmuls are bf16 with fp32
accumulation; softmax skips max-subtraction (logits ~N(0,1), exp safe in fp32).
"""

import ml_dtypes
import numpy as np

import concourse.bass as bass
import concourse.mybir as mybir
import concourse.tile as tile
from concourse.bass_utils import run_bass_kernel_spmd

B = 2
N = 2048
D = 1024
H = 16
DH = 64
HPC = 4  # heads per core
NCORES = 8
HB = HPC * DH  # 256: head-block width per core
NKT = N // 128  # 16 k-tiles
NW = 4  # 512-wide q-windows

f32 = mybir.dt.float32
f32r = mybir.dt.float32r
bf16 = mybir.dt.bfloat16
EXP = mybir.ActivationFunctionType.Exp

_WAIT_CAP = 1


def _split_excess_waits(nc):
    """The walrus build in this container rejects instructions carrying more
    than a couple of sync-wait commands ("Too many sync wait commands" in
    CoreV3GenImpl setupSyncWait). Tile's semaphore assignment freely attaches
    several waits to one instruction. Hoist the excess onto dedicated
    single-wait NOPs inserted just before the instruction on the same engine
    (program order on that engine preserves the wait-before-execute
    semantics)."""
    f = nc.m.functions[0]
    for blk in f.blocks:
        out = []
        changed = False
        for inst in blk.instructions:
            si = inst.sync_info
            waits = list(si.on_wait) if si is not None and si.on_wait else []
            if len(waits) > _WAIT_CAP:
                changed = True
                for j, w in enumerate(waits[: -_WAIT_CAP]):
                    nop = mybir.InstNoOp(
                        name=f"{inst.name}-ws{j}",
                        engine=inst.engine,
                        sync_info=mybir.SyncInfo(on_wait=[w], on_update=[]),
                        bass_nofuse=True,
                    )
                    nc.register_instruction(nop)
                    out.append(nop)
                si.on_wait = waits[-_WAIT_CAP:]
            out.append(inst)
        if changed:
            blk.instructions = out


def _r(ap):
    return ap.bitcast(f32r)


def _build_nc():
    nc = bass.Bass()
    xT_d = nc.dram_tensor("xT", [D, N], bf16, kind="ExternalInput")
    wqT_d = nc.dram_tensor("wqT", [D, HB], bf16, kind="ExternalInput")
    wkT_d = nc.dram_tensor("wkT", [D, HB], bf16, kind="ExternalInput")
    wvT_d = nc.dram_tensor("wvT", [D, HB], bf16, kind="ExternalInput")
    woT_d = nc.dram_tensor("woT", [HB, D], bf16, kind="ExternalInput")
    y_ds = [
        nc.dram_tensor(f"y{it}", [N, D], f32, kind="ExternalOutput") for it in range(2)
    ]

    with tile.TileContext(nc) as tc:
        with (
            tc.tile_pool(name="main", bufs=1) as main,
            tc.tile_pool(name="ptp", bufs=3) as ptp,
            tc.tile_pool(name="ysp", bufs=3) as ysp,
            tc.tile_pool(name="spp", bufs=2) as spp,
            tc.tile_pool(name="drm", bufs=2, space="DRAM") as drm,
            tc.tile_pool(name="aux", bufs=1, space="PSUM") as aux,
            tc.tile_pool(name="stp", bufs=2, space="PSUM") as stp,
            tc.tile_pool(name="otp", bufs=1, space="PSUM") as otp,
        ):
            # persistent tensors
            qT = main.tile([128, 2, N], bf16)  # row d = it*128+p
            kT = main.tile([128, 2, N], bf16)
            vaug = main.tile([128, NKT, HPC, DH + 1], bf16)  # [k%128, k//128, h, d|1]
            ocat = main.tile([128, 2, N], bf16)  # row i = it*128+p
            sel = main.tile([2, 128], f32)
            wo = main.tile([128, 2, D], bf16)
            xt = main.tile([128, 8, N], bf16)
            wq = main.tile([128, 8, HB], bf16)
            wk = main.tile([128, 8, HB], bf16)
            wv = main.tile([128, 8, HB], bf16)

            sel_np = np.zeros((2, 128), dtype=np.float32)
            sel_np[0, 0:64] = 1.0
            sel_np[1, 64:128] = 1.0
            sel_d = nc.inline_tensor(sel_np, name="sel_const")
            nc.sync.dma_start(_r(sel[:]), _r(sel_d[:]))
            ones_t = main.tile([128, 1], bf16)
            nc.vector.memset(ones_t[:], 1.0)
            nc.vector.tensor_copy(
                vaug[:, :, :, DH : DH + 1],
                ones_t[:, :, None, None].to_broadcast([128, NKT, HPC, 1]),
            )
            for eo in range(8):
                sl = slice(eo * 128, (eo + 1) * 128)
                nc.sync.dma_start(wk[:, eo], wkT_d[sl])
                nc.gpsimd.dma_start(wq[:, eo], wqT_d[sl])
            for eo in range(8):
                sl = slice(eo * 128, (eo + 1) * 128)
                nc.sync.dma_start(xt[:, eo], xT_d[sl])
                nc.gpsimd.dma_start(wv[:, eo], wvT_d[sl])
            for it in range(2):
                nc.gpsimd.dma_start(wo[:, it], woT_d[it * 128 : (it + 1) * 128])

            # ---- projection-group emitters (each: 8 accumulating matmuls) ----
            def emit_qk_group(dst, w, it, q4):
                ps = aux.tile([128, 512], f32, tag="qkv")
                for eo in range(8):
                    nc.tensor.matmul(
                        ps,
                        lhsT=w[:, eo, it * 128 : (it + 1) * 128],
                        rhs=xt[:, eo, q4 * 512 : (q4 + 1) * 512],
                        start=(eo == 0),
                        stop=(eo == 7),
                    )
                nc.vector.tensor_copy(dst[:, it, q4 * 512 : (q4 + 1) * 512], ps)

            def emit_v_group(nt):
                ps_full = aux.tile([128, 512], f32, tag="qkv", name=f"vps{nt}")
                ps = ps_full[:, 0:HB]
                for eo in range(8):
                    nc.tensor.matmul(
                        ps,
                        lhsT=xt[:, eo, nt * 128 : (nt + 1) * 128],
                        rhs=wv[:, eo, :],
                        start=(eo == 0),
                        stop=(eo == 7),
                    )
                nc.vector.tensor_copy(
                    vaug[:, nt, :, 0:DH], ps.rearrange("p (h d) -> p h d", h=HPC)
                )

            # upfront groups: what the first attention step (pair 0, window 0)
            # consumes early: kT it0 (read across all k-tiles) and qT it0 w0.
            emit_qk_group(kT, wk, 0, 0)
            emit_qk_group(qT, wq, 0, 0)

            # remaining projection groups, drip-fed into attention steps at
            # ~1 group per 5 k-tiles so the exp stream never starves and the
            # single qkv PSUM slot never backs up. Tile tracks dependencies in
            # emission order, so every group is EMITTED strictly before its
            # consumer (step s consumes qT[it=s//4] window q4=s%4 and, within
            # its own kt loop, kT[it] window q4=kt//4; kT it1 from step 4 on).
            def qk(dst, w, it, q4):
                return lambda: emit_qk_group(dst, w, it, q4)

            drip = {
                0: {2: qk(kT, wk, 0, 1), 6: qk(kT, wk, 0, 2), 10: qk(kT, wk, 0, 3),
                    13: qk(qT, wq, 0, 1)},
                1: {1: qk(qT, wq, 0, 2), 6: qk(kT, wk, 1, 0), 11: qk(kT, wk, 1, 1)},
                2: {1: qk(qT, wq, 0, 3), 6: qk(kT, wk, 1, 2), 11: qk(kT, wk, 1, 3)},
                3: {1: qk(qT, wq, 1, 0), 8: qk(qT, wq, 1, 1)},
                4: {4: qk(qT, wq, 1, 2)},
                5: {4: qk(qT, wq, 1, 3)},
            }

            # ---- attention steps ----
            steps = [(it, w) for it in range(2) for w in range(NW)]

            def att_step(step_idx, it, w, mid=None):
                q0 = w * 512
                h_lo, h_hi = 2 * it, 2 * it + 1
                ot_lo = otp.tile([128, 512], f32, tag="otlo")
                ot_hi = otp.tile([128, 512], f32, tag="othi")
                fillers = dict(drip.get(step_idx, {}))
                for kt in range(NKT):
                    if mid is not None and kt == 6:
                        mid()
                        mid = None
                    if step_idx == 0:
                        emit_v_group(kt)
                    f = fillers.pop(kt, None)
                    if f is not None:
                        f()
                    st2 = stp.tile([128, 2, 512], f32, tag="st")
                    for s in range(2):
                        nc.tensor.matmul(
                            st2[:, s, :],
                            lhsT=kT[
                                s * 64 : s * 64 + 64, it, kt * 128 : (kt + 1) * 128
                            ],
                            rhs=qT[s * 64 : s * 64 + 64, it, q0 : q0 + 512],
                            start=True,
                            stop=True,
                        )
                    pt2 = ptp.tile([128, 2, 512], bf16, tag="pt")
                    nc.scalar.activation(
                        pt2.rearrange("p s q -> p (s q)"),
                        st2.rearrange("p s q -> p (s q)"),
                        EXP,
                        scale=0.125,
                    )
                    for s, ot in ((0, ot_lo), (1, ot_hi)):
                        nc.tensor.matmul(
                            ot[0:65, :],
                            lhsT=vaug[:, kt, 2 * it + s, :],
                            rhs=pt2[:, s, :],
                            start=(kt == 0),
                            stop=(kt == NKT - 1),
                        )
                if mid is not None:
                    mid()
                for kt in sorted(fillers):
                    fillers[kt]()
                # evacuate: unnormalized O^T (cast to bf16) + row sums
                sst = spp.tile([1, 1024], f32, tag="sst")
                for s, ot in ((0, ot_lo), (1, ot_hi)):
                    nc.vector.tensor_copy(
                        ocat[s * 64 : s * 64 + 64, it, q0 : q0 + 512], ot[0:64, :]
                    )
                    nc.vector.tensor_copy(
                        sst[0:1, s * 512 : (s + 1) * 512], ot[64:65, :]
                    )
                return sst

            def np_step(it, w, sst):
                q0 = w * 512
                # reciprocal of the row sums, spread over 128 partitions via a
                # DRAM bounce (SBUF partition dim can't be reshaped in place)
                stmp = drm.tile([1, 1024], f32, tag="stmp")
                nc.sync.dma_start(stmp, sst)
                spk = spp.tile([128, 8], f32, tag="spk")
                nc.sync.dma_start(spk, stmp.rearrange("a (p j) -> (a p) j", p=128))
                rpk = spp.tile([128, 8], f32, tag="rpk")
                nc.vector.reciprocal(rpk, spk)
                rtmp = drm.tile([128, 8], f32, tag="rtmp")
                nc.sync.dma_start(rtmp, rpk)
                rq = spp.tile([2, 512], f32, tag="rq")
                nc.sync.dma_start(
                    _r(rq[:]),
                    _r(rtmp.rearrange("p j -> (p j)").rearrange("(s q) -> s q", s=2)),
                )
                # broadcast 1/sum across each head's 64 rows and normalize
                bc = aux.tile([128, 512], f32, tag="np")
                nc.tensor.matmul(bc, lhsT=_r(sel), rhs=_r(rq), start=True, stop=True)
                bcb = ysp.tile([128, 512], bf16, tag="bcb")
                nc.vector.tensor_copy(bcb[:], bc)
                osl = ocat[:, it, q0 : q0 + 512]
                nc.vector.tensor_mul(osl, osl, bcb)
                # output projection for this (i-tile, window): partial sums
                for qt in range(4):
                    r0 = q0 + qt * 128
                    for oc in range(2):
                        yp = aux.tile([128, 512], f32, tag="np")
                        nc.tensor.matmul(
                            yp,
                            lhsT=ocat[:, it, r0 : r0 + 128],
                            rhs=wo[:, it, oc * 512 : (oc + 1) * 512],
                            start=True,
                            stop=True,
                        )
                        ys = ysp.tile([128, 512], f32, tag="ys")
                        nc.vector.tensor_copy(ys, yp)
                        nc.sync.dma_start(
                            y_ds[it][r0 : r0 + 128, oc * 512 : (oc + 1) * 512], ys
                        )

            prev_np = None  # previous step's norm+project, emitted mid next step
            for idx, (it, w) in enumerate(steps):
                sst = att_step(idx, it, w, mid=prev_np)
                prev_np = (
                    lambda it=it, w=w, sst=sst: np_step(it, w, sst)
                )
            prev_np()  # the last step's norm+project is the kernel tail

    _split_excess_waits(nc)
    return nc


_CACHED_NC = None


def _get_nc():
    global _CACHED_NC
    if _CACHED_NC is None:
        _CACHED_NC = _build_nc()
    return _CACHED_NC


def _make_in_maps(x, w_qkv):
    b16 = ml_dtypes.bfloat16

    def c(a):
        return np.ascontiguousarray(a.astype(b16))

    in_maps = []
    xT = [c(x[b].T) for b in range(B)]
    for core in range(NCORES):
        b = core // (NCORES // B)
        hb = core % (NCORES // B)
        rows = slice(hb * HB, (hb + 1) * HB)
        wq = c(w_qkv[0 * D : 1 * D][rows].T)
        wk = c(w_qkv[1 * D : 2 * D][rows].T)
        wv = c(w_qkv[2 * D : 3 * D][rows].T)
        in_maps.append({"xT": xT[b], "wqT": wq, "wkT": wk, "wvT": wv})
    return in_maps


def kernel(x, w_qkv, w_out, b_out, _trace=False, _trace_kwargs=None):
    x = np.asarray(x, dtype=np.float32)
    w_qkv = np.asarray(w_qkv, dtype=np.float32)
    w_out = np.asarray(w_out, dtype=np.float32)
    b_out = np.asarray(b_out, dtype=np.float32)

    in_maps = _make_in_maps(x, w_qkv)
    for core in range(NCORES):
        hb = core % (NCORES // B)
        woT = np.ascontiguousarray(
            w_out[:, hb * HB : (hb + 1) * HB].T.astype(ml_dtypes.bfloat16)
        )
        in_maps[core]["woT"] = woT

    nc = _get_nc()
    kwargs = {}
    if _trace:
        kwargs["trace"] = True
        if _trace_kwargs:
            kwargs.update(_trace_kwargs)
    res = run_bass_kernel_spmd(nc, in_maps, core_ids=list(range(NCORES)), **kwargs)

    out = np.zeros((B, N, D), dtype=np.float32)
    for core in range(NCORES):
        b = core // (NCORES // B)
        out[b] += res.results[core]["y0"]
        out[b] += res.results[core]["y1"]
    out += b_out[None, None, :]
    kernel._last_result = res
    return out
